# revision 1
# baseline (speedup 1.0000x reference)
"""Trainium2 Bass kernel for nn_AttentionBlock_9792525435528.

Reference computation (per batch element b):
    xf = x[b].reshape(C, T)                      # C=512, T=32*32=1024
    GroupNorm(G=32) -> xn
    qkv = qkv_w @ xn + qkv_b                     # [3C, T]
    per head h (NH=8, ch=64): q,k,v; w = softmax((q*s)^T (k*s)); a = v @ w^T
    h = proj_w @ a + proj_b
    out = (xf + h) / sqrt(2)

Sharding: data-parallel over batch. 8 batch elements -> 8 NeuronCores, one
each. Weights replicated. No cross-core communication needed.

Device algorithm highlights:
  - float32r matmuls (full-rate PE, no data conversion; inputs stay fp32).
  - GroupNorm stats: bn_stats/bn_aggr per channel, then two tiny PE matmuls
    against constant indicator matrices to reduce across the 16 channels of
    each group and broadcast (mu, rstd) back to per-channel partitions.
  - Attention computed in the w^T[s,t] layout (softmax dim s on partitions):
    no max-subtraction needed (logits are O(1) by construction), exp runs on
    ScalarE straight out of PSUM into bf16 SBUF tiles, and the softmax
    denominator comes from an all-ones lhsT matmul col-tiled next to the
    a-matmul (both accumulate over s concurrently in different PE column
    groups). Division by the denominator uses reciprocal_approx_fast.
  - V is produced already transposed (v^T[s, c]) by swapping the matmul
    operands (lhsT = xn), so no on-device transpose is ever required.
  - q/k scale (1/sqrt(sqrt(ch))) and the final 1/sqrt(2) are folded into the
    weights on the host.
"""

import ml_dtypes
import numpy as np

import concourse.bass as bass
import concourse.mybir as mybir
import concourse.tile as tile
from concourse import bacc
from concourse.bass_utils import run_bass_kernel_spmd

B, C, T = 8, 512, 1024
NH, CH, G = 8, 64, 32
GS = C // G  # 16 channels per group
EPS = 1e-6
NCORES = 8
P = 128
KC = C // P  # 4 chunks of 128 input channels
SCN = T // P  # 8 s-chunks
NT = T // 512  # 2 t-chunks of 512
ISQ2 = float(1.0 / np.sqrt(2.0))
QK_SCALE = float(1.0 / np.sqrt(np.sqrt(CH)))

F32 = mybir.dt.float32
F32R = mybir.dt.float32r
BF16 = mybir.dt.bfloat16

_GRAPH_CACHE = {}


def _build_graph(qkv_bias_nz: bool, proj_bias_nz: bool, use_f32r: bool = True,
                 debug_taps: bool = False):
    nc = bacc.Bacc("TRN2", target_bir_lowering=False, debug=False)
    # All large matmuls run in bf16: 1 cycle/row on the PE (fp32/f32r run at
    # 2 cyc/row via the fp32_mode=HIGH path) plus fast weight load. PSUM
    # accumulation stays fp32; the residual path stays fp32 end-to-end, so
    # the bf16 rounding lands well inside the 2e-2 tolerance.
    MMD = BF16

    # ---- DRAM I/O ------------------------------------------------------
    x_d = nc.dram_tensor("x", [C, T], F32, kind="ExternalInput").ap()
    wq_d = nc.dram_tensor("wqT", [C, C], MMD, kind="ExternalInput").ap()
    wk_d = nc.dram_tensor("wkT", [C, C], MMD, kind="ExternalInput").ap()
    wv_d = nc.dram_tensor("wvT", [C, C], MMD, kind="ExternalInput").ap()
    pw_d = nc.dram_tensor("pwT", [C, C], MMD, kind="ExternalInput").ap()
    gnw_d = nc.dram_tensor("gnw", [C], F32, kind="ExternalInput").ap()
    gnb_d = nc.dram_tensor("gnb", [C], F32, kind="ExternalInput").ap()
    ind16_d = nc.dram_tensor("ind16", [C, G], F32, kind="ExternalInput").ap()
    indT_d = nc.dram_tensor("indT", [G, C], F32, kind="ExternalInput").ap()
    qb_d = kb_d = vb_d = pb_d = None
    if qkv_bias_nz:
        qb_d = nc.dram_tensor("qb", [C], F32, kind="ExternalInput").ap()
        kb_d = nc.dram_tensor("kb", [C], F32, kind="ExternalInput").ap()
        vb_d = nc.dram_tensor("vb", [C], F32, kind="ExternalInput").ap()
    if proj_bias_nz:
        pb_d = nc.dram_tensor("pb", [C], F32, kind="ExternalInput").ap()
    out_d = nc.dram_tensor("out", [C, T], F32, kind="ExternalOutput").ap()
    dbg = {}
    if debug_taps:
        MMDn = MMD
        dbg["xn"] = nc.dram_tensor("dbg_xn", [P, KC, T], MMDn, kind="ExternalOutput").ap()
        dbg["q"] = nc.dram_tensor("dbg_q", [P, KC, T], MMDn, kind="ExternalOutput").ap()
        dbg["k"] = nc.dram_tensor("dbg_k", [P, KC, T], MMDn, kind="ExternalOutput").ap()
        dbg["vt"] = nc.dram_tensor("dbg_vt", [P, SCN, NH * P], BF16, kind="ExternalOutput").ap()
        dbg["ew0"] = nc.dram_tensor("dbg_ew0", [SCN, P, T], BF16, kind="ExternalOutput").ap()
        dbg["a"] = nc.dram_tensor("dbg_a", [P, KC, T], MMDn, kind="ExternalOutput").ap()

    with tile.TileContext(nc) as tc:
        with (
            tc.tile_pool(name="big", bufs=1) as big,
            tc.tile_pool(name="wpool", bufs=1) as wpool,
            tc.tile_pool(name="small", bufs=1) as small,
            tc.tile_pool(name="ew", bufs=36) as ewpool,
            tc.tile_pool(name="rcp", bufs=8) as rcpool,
            tc.tile_pool(name="ps1", bufs=2, space="PSUM") as ps1,
            tc.tile_pool(name="ps2", bufs=2, space="PSUM") as ps2,
            tc.tile_pool(name="psg", bufs=2, space="PSUM") as psg,
        ):
            # ---- load inputs ------------------------------------------
            x_sb = big.tile([P, KC, T], F32, tag="x")
            x_dr = x_d.rearrange("(o p) t -> p o t", p=P)
            for o in range(KC):
                nc.sync.dma_start(out=x_sb[:, o, :], in_=x_dr[:, o, :])

            wq_sb = wpool.tile([P, KC, C], MMD, tag="wq")
            nc.sync.dma_start(out=wq_sb, in_=wq_d.rearrange("(o p) n -> p o n", p=P))
            wk_sb = wpool.tile([P, KC, C], MMD, tag="wk")
            nc.sync.dma_start(out=wk_sb, in_=wk_d.rearrange("(o p) n -> p o n", p=P))
            wv_sb = wpool.tile([P, KC, C], MMD, tag="wv")
            nc.sync.dma_start(out=wv_sb, in_=wv_d.rearrange("(o p) n -> p o n", p=P))
            pw_sb = wpool.tile([P, KC, C], MMD, tag="pw")
            nc.sync.dma_start(out=pw_sb, in_=pw_d.rearrange("(o p) n -> p o n", p=P))

            gnw_sb = small.tile([P, KC], F32, tag="gnw")
            nc.sync.dma_start(out=gnw_sb, in_=gnw_d.rearrange("(o p) -> p o", p=P))
            gnb_sb = small.tile([P, KC], F32, tag="gnb")
            nc.sync.dma_start(out=gnb_sb, in_=gnb_d.rearrange("(o p) -> p o", p=P))
            ind16_sb = small.tile([P, KC, G], F32, tag="ind16")
            nc.sync.dma_start(
                out=ind16_sb, in_=ind16_d.rearrange("(o p) g -> p o g", p=P)
            )
            indT_sb = small.tile([G, KC, P], F32, tag="indT")
            nc.sync.dma_start(out=indT_sb, in_=indT_d.rearrange("g (o p) -> g o p", p=P))

            bias_aps = {}
            for nm, d in (("qb", qb_d), ("kb", kb_d), ("pb", pb_d)):
                if d is not None:
                    t_ = small.tile([P, KC], F32, tag=nm)
                    nc.sync.dma_start(out=t_, in_=d.rearrange("(o p) -> p o", p=P))
                    bias_aps[nm] = t_
            if vb_d is not None:
                # v-bias varies along the free dim of v^T tiles: broadcast to
                # all 128 partitions once.
                vb_bc = small.tile([P, C], F32, tag="vb")
                nc.sync.dma_start(
                    out=vb_bc,
                    in_=bass.AP(tensor=vb_d.tensor, offset=vb_d.offset,
                                ap=[[0, P]] + vb_d.ap),
                )
                bias_aps["vb"] = vb_bc


            # Warm the ScalarE table set (Ln/Exp) while the x DMA is in
            # flight, so the ~2.7us ACT_TABLE_LOAD is off the critical path.
            warm = small.tile([G, 1], F32, tag="warm")
            nc.vector.memset(warm, 1.0)
            nc.scalar.activation(
                out=warm, in_=warm, func=mybir.ActivationFunctionType.Exp
            )

            # ---- GroupNorm statistics ---------------------------------
            stats6 = small.tile([P, KC, 2, 6], F32, tag="stats6")
            mv = small.tile([P, KC, 2], F32, tag="mv")
            stats2 = small.tile([P, KC, 2], F32, tag="stats2")
            for o in range(KC):
                for hlf in range(2):
                    nc.vector.bn_stats(
                        out=stats6[:, o, hlf, :],
                        in_=x_sb[:, o, hlf * 512:(hlf + 1) * 512],
                    )
                nc.vector.bn_aggr(out=mv[:, o, :], in_=stats6[:, o, :, :])
                # stats2 = (mean, E[x^2]) per channel
                nc.vector.tensor_copy(out=stats2[:, o, 0:1], in_=mv[:, o, 0:1])
                nc.vector.tensor_mul(
                    out=stats2[:, o, 1:2], in0=mv[:, o, 0:1], in1=mv[:, o, 0:1]
                )
                nc.vector.tensor_add(
                    out=stats2[:, o, 1:2], in0=stats2[:, o, 1:2], in1=mv[:, o, 1:2]
                )

            # group reduce: psum_s[g, :] = (mu_g, E[x^2]_g)  (ind16 holds 1/16)
            psum_s = psg.tile([G, 2], F32, tag="psg")
            for k in range(KC):
                nc.tensor.matmul(
                    psum_s,
                    lhsT=ind16_sb[:, k, :],
                    rhs=stats2[:, k, :],
                    start=(k == 0),
                    stop=(k == KC - 1),
                )
            musd = small.tile([G, 2], F32, tag="musd")
            # musd[:,0] = mu ; musd[:,1] = rstd = exp(-0.5*ln(var+eps))
            nc.vector.tensor_copy(out=musd, in_=psum_s)  # (mu, E[x^2])
            varg = small.tile([G, 1], F32, tag="varg")
            nc.vector.tensor_mul(out=varg, in0=musd[:, 0:1], in1=musd[:, 0:1])
            nc.vector.tensor_sub(out=varg, in0=musd[:, 1:2], in1=varg)
            eps_sb = small.tile([G, 1], F32, tag="eps")
            nc.vector.memset(eps_sb, EPS)
            nc.scalar.activation(
                out=varg, in_=varg, func=mybir.ActivationFunctionType.Ln, bias=eps_sb
            )
            nc.scalar.activation(
                out=musd[:, 1:2], in_=varg,
                func=mybir.ActivationFunctionType.Exp, scale=-0.5,
            )

            # broadcast (mu, rstd) back to per-channel layout [P, KC, 2]
            musd_c = small.tile([P, KC, 2], F32, tag="musd_c")
            for o in range(KC):
                psum_b = psg.tile([P, 2], F32, tag="psg")
                nc.tensor.matmul(
                    psum_b, lhsT=indT_sb[:, o, :], rhs=musd, start=True, stop=True
                )
                nc.vector.tensor_copy(out=musd_c[:, o, :], in_=psum_b)

            # A = rstd * gn_w ; Bq = gn_b - mu * A   (per channel)
            A_sb = small.tile([P, KC], F32, tag="A")
            B_sb = small.tile([P, KC], F32, tag="B")
            for o in range(KC):
                nc.vector.tensor_mul(
                    out=A_sb[:, o:o + 1], in0=musd_c[:, o, 1:2], in1=gnw_sb[:, o:o + 1]
                )
                nc.vector.tensor_mul(
                    out=B_sb[:, o:o + 1], in0=musd_c[:, o, 0:1], in1=A_sb[:, o:o + 1]
                )
                nc.vector.tensor_sub(
                    out=B_sb[:, o:o + 1], in0=gnb_sb[:, o:o + 1], in1=B_sb[:, o:o + 1]
                )

            # xn = x * A + B
            xn_sb = big.tile([P, KC, T], MMD, tag="xn")
            for o in range(KC):
                nc.vector.tensor_scalar(
                    out=xn_sb[:, o, :], in0=x_sb[:, o, :],
                    scalar1=A_sb[:, o:o + 1], scalar2=B_sb[:, o:o + 1],
                    op0=mybir.AluOpType.mult, op1=mybir.AluOpType.add,
                )

            # ---- QKV projections --------------------------------------
            # q_sb/k_sb: [P, pair, T]; rows 0:64 = head 2j, 64:128 = head 2j+1
            q_sb = big.tile([P, KC, T], MMD, tag="q")
            k_sb = big.tile([P, KC, T], MMD, tag="k")
            for dst, w_sb, bias in (
                (q_sb, wq_sb, bias_aps.get("qb")),
                (k_sb, wk_sb, bias_aps.get("kb")),
            ):
                for j in range(KC):  # head pair
                    for t in range(NT):
                        pq = psg.tile([P, 512], F32, tag="psg")
                        for k in range(KC):
                            nc.tensor.matmul(
                                pq,
                                lhsT=w_sb[:, k, j * P:(j + 1) * P],
                                rhs=xn_sb[:, k, t * 512:(t + 1) * 512],
                                start=(k == 0),
                                stop=(k == KC - 1),
                            )
                        dslice = dst[:, j, t * 512:(t + 1) * 512]
                        if bias is not None:
                            nc.vector.tensor_scalar(
                                out=dslice, in0=pq, scalar1=bias[:, j:j + 1],
                                scalar2=None, op0=mybir.AluOpType.add,
                            )
                        else:
                            nc.vector.tensor_copy(out=dslice, in_=pq)

            # v^T augmented: per head 128 cols = [64 v^T cols | 64 ones].
            # MM2's lhsT is then [s, 128]: rows 0:64 of its PSUM output get
            # sum_s v^T*ew (attention numerator) and rows 64:128 get
            # sum_s ew (softmax denominator) in a single accumulation group.
            vT_sb = big.tile([P, SCN, NH * P], BF16, tag="vT")
            vT4 = vT_sb.rearrange("p s (h z) -> p s h z", z=P)
            nc.gpsimd.memset(vT4[:, :, :, CH:P], 1.0)
            for sc in range(SCN):
                pv = psg.tile([P, 512], F32, tag="psg")
                for k in range(KC):
                    nc.tensor.matmul(
                        pv,
                        lhsT=xn_sb[:, k, sc * P:(sc + 1) * P],
                        rhs=wv_sb[:, k, :],
                        start=(k == 0),
                        stop=(k == KC - 1),
                    )
                vdst = vT4[:, sc, :, 0:CH]  # [P, NH, CH] strided dst
                if "vb" in bias_aps:
                    nc.vector.scalar_tensor_tensor(
                        out=vdst, in0=pv.rearrange("p (h z) -> p h z", z=CH),
                        scalar=0.0,
                        in1=bias_aps["vb"].rearrange("p (h z) -> p h z", z=CH),
                        op0=mybir.AluOpType.add, op1=mybir.AluOpType.add,
                    )
                else:
                    nc.vector.tensor_copy(
                        out=vdst, in_=pv.rearrange("p (h z) -> p h z", z=CH)
                    )

            if debug_taps:
                nc.sync.dma_start(out=dbg["xn"], in_=xn_sb)
                nc.sync.dma_start(out=dbg["q"], in_=q_sb)
                nc.sync.dma_start(out=dbg["k"], in_=k_sb)
                nc.sync.dma_start(out=dbg["vt"], in_=vT_sb)

            # ---- attention + proj rhs ---------------------------------
            # Software-pipelined over head pairs: MM2 of pair j-1 is emitted
            # after MM1+exp of pair j, so the PE has ready matmul work while
            # ScalarE works through pair j's exp queue (the attention phase
            # is ScalarE-bound per pair).
            a_sb = big.tile([P, KC, T], MMD, tag="a")

            def emit_mm1(j, ew):
                for sc in range(SCN):
                    ptiles = {}
                    for hb in range(2):  # row-group-tiled head pair
                        h0 = hb * CH
                        pw1 = ps1.tile([P, T], F32, tag="ps1")
                        for t in range(NT):
                            nc.tensor.matmul(
                                pw1[:, t * 512:(t + 1) * 512],
                                lhsT=k_sb[h0:h0 + CH, j, sc * P:(sc + 1) * P],
                                rhs=q_sb[h0:h0 + CH, j, t * 512:(t + 1) * 512],
                                start=True,
                                stop=True,
                            )
                        ptiles[hb] = pw1
                    for hb in range(2):
                        et = ewpool.tile([P, T], BF16, tag="ew")
                        nc.scalar.activation(
                            out=et, in_=ptiles[hb],
                            func=mybir.ActivationFunctionType.Exp,
                        )
                        ew[(hb, sc)] = et
                        if debug_taps and j == 0 and hb == 0:
                            nc.sync.dma_start(out=dbg["ew0"][sc], in_=et)

            def emit_mm2(j, ew):
                for hb in range(2):
                    h = 2 * j + hb
                    for t in range(NT):
                        pa = ps2.tile([P, 512], F32, tag="ps2")
                        for sc in range(SCN):
                            # rows 0:64 <- sum_s v^T[s,c]*ew[s,t]
                            # rows 64:128 <- sum_s ew[s,t] (softmax denom)
                            nc.tensor.matmul(
                                pa,
                                lhsT=vT_sb[:, sc, h * P:(h + 1) * P],
                                rhs=ew[(hb, sc)][:, t * 512:(t + 1) * 512],
                                start=(sc == 0),
                                stop=(sc == SCN - 1),
                            )
                        # reciprocal_approx_fast cannot read PSUM (HW-
                        # verified): stage the denominator into SBUF first.
                        d_sb = rcpool.tile([CH, 512], F32, tag="dcp")
                        nc.vector.tensor_copy(out=d_sb, in_=pa[CH:2 * CH, :])
                        r_sb = rcpool.tile([CH, 512], F32, tag="rcp")
                        nc.vector.reciprocal_approx_fast(out=r_sb, in_=d_sb)
                        nc.vector.tensor_mul(
                            out=a_sb[hb * CH:(hb + 1) * CH, j,
                                     t * 512:(t + 1) * 512],
                            in0=pa[0:CH, :],
                            in1=r_sb,
                        )

            ew_prev = None
            for j in range(KC):  # head pairs (2j, 2j+1)
                ew_cur = {}
                emit_mm1(j, ew_cur)
                if ew_prev is not None:
                    emit_mm2(j - 1, ew_prev)
                ew_prev = ew_cur
            emit_mm2(KC - 1, ew_prev)

            # ---- output projection + residual -------------------------
            out_sb = big.tile([P, KC, T], F32, tag="osb")
            for o in range(KC):
                for t in range(NT):
                    ph = psg.tile([P, 512], F32, tag="psg")
                    for k in range(KC):
                        nc.tensor.matmul(
                            ph,
                            lhsT=pw_sb[:, k, o * P:(o + 1) * P],
                            rhs=a_sb[:, k, t * 512:(t + 1) * 512],
                            start=(k == 0),
                            stop=(k == KC - 1),
                        )
                    if "pb" in bias_aps:
                        nc.vector.tensor_scalar(
                            out=ph, in0=ph, scalar1=bias_aps["pb"][:, o:o + 1],
                            scalar2=None, op0=mybir.AluOpType.add,
                        )
                    # out = x * (1/sqrt2) + h'   (1/sqrt2 folded into pwT/pb)
                    nc.vector.scalar_tensor_tensor(
                        out=out_sb[:, o, t * 512:(t + 1) * 512],
                        in0=x_sb[:, o, t * 512:(t + 1) * 512],
                        scalar=ISQ2,
                        in1=ph,
                        op0=mybir.AluOpType.mult,
                        op1=mybir.AluOpType.add,
                    )
                # stream each o-chunk out as soon as its epilogue is done
                nc.sync.dma_start(
                    out=out_d.rearrange("(o p) t -> p o t", p=P)[:, o, :],
                    in_=out_sb[:, o, :],
                )

    nc.compile()
    return nc


def _host_prep(qkv_w, qkv_b, proj_w, proj_b):
    """Build the replicated (per-core-identical) weight/const arrays."""
    qkv_w = np.asarray(qkv_w, np.float32)
    qkv_b = np.asarray(qkv_b, np.float32)
    proj_w = np.asarray(proj_w, np.float32)
    proj_b = np.asarray(proj_b, np.float32)

    w3 = qkv_w.reshape(NH, 3 * CH, C)  # per head: rows 0:64 q, 64:128 k, 128:192 v
    b3 = qkv_b.reshape(NH, 3 * CH)
    wq = w3[:, 0:CH, :] * QK_SCALE          # [NH, CH, C]
    wk = w3[:, CH:2 * CH, :] * QK_SCALE
    wv = w3[:, 2 * CH:3 * CH, :]
    qb = (b3[:, 0:CH] * QK_SCALE).reshape(C)
    kb = (b3[:, CH:2 * CH] * QK_SCALE).reshape(C)
    vb = b3[:, 2 * CH:3 * CH].reshape(C)

    # lhsT layouts [C_in, C_out-ish]: column r of pair-block j is head 2j's q
    # row r (r<64) or head 2j+1's q row r-64.
    BF = ml_dtypes.bfloat16
    wqT = np.ascontiguousarray(wq.reshape(C, C).T.astype(BF))  # [C_in, NH*CH]
    wkT = np.ascontiguousarray(wk.reshape(C, C).T.astype(BF))
    wvT = np.ascontiguousarray(wv.reshape(C, C).T.astype(BF))
    pwT = np.ascontiguousarray((proj_w * ISQ2).T.astype(BF))
    pb = proj_b * ISQ2

    # per-partition bias layouts for q/k ([C] ordered head-major == qkv order)
    ind16 = np.zeros((C, G), np.float32)
    ind16[np.arange(C), np.arange(C) // GS] = 1.0 / GS
    indT = np.zeros((G, C), np.float32)
    indT[np.arange(C) // GS, np.arange(C)] = 1.0

    return dict(
        wqT=wqT, wkT=wkT, wvT=wvT, pwT=pwT,
        qb=qb, kb=kb, vb=vb, pb=pb,
        ind16=ind16, indT=indT,
    )


def kernel(**inputs):
    x = np.asarray(inputs["x"], np.float32)
    gn_w = np.asarray(inputs["gn_w"], np.float32)
    gn_b = np.asarray(inputs["gn_b"], np.float32)
    qkv_b = np.asarray(inputs["qkv_b"], np.float32)
    proj_b = np.asarray(inputs["proj_b"], np.float32)

    prep = _host_prep(inputs["qkv_w"], qkv_b, inputs["proj_w"], proj_b)
    qkv_bias_nz = bool(np.any(qkv_b != 0))
    proj_bias_nz = bool(np.any(proj_b != 0))

    key = (qkv_bias_nz, proj_bias_nz)
    if key not in _GRAPH_CACHE:
        _GRAPH_CACHE[key] = _build_graph(qkv_bias_nz, proj_bias_nz)
    nc = _GRAPH_CACHE[key]

    shared = dict(
        wqT=prep["wqT"], wkT=prep["wkT"], wvT=prep["wvT"], pwT=prep["pwT"],
        gnw=gn_w, gnb=gn_b, ind16=prep["ind16"], indT=prep["indT"],
    )
    if qkv_bias_nz:
        shared.update(qb=prep["qb"], kb=prep["kb"], vb=prep["vb"])
    if proj_bias_nz:
        shared.update(pb=prep["pb"])

    in_maps = [
        {**shared, "x": np.ascontiguousarray(x[i].reshape(C, T))}
        for i in range(NCORES)
    ]
    res = run_bass_kernel_spmd(nc, in_maps, core_ids=list(range(NCORES)))
    out = np.stack(
        [res.results[i]["out"].reshape(C, 32, 32) for i in range(NCORES)]
    )
    kernel._last_results = res
    return out



# revision 5
# speedup vs baseline: 1.1023x; 1.1023x over previous
"""Trainium2 Bass kernel for nn_AttentionBlock_9792525435528.

Reference computation (per batch element b):
    xf = x[b].reshape(C, T)                      # C=512, T=32*32=1024
    GroupNorm(G=32) -> xn
    qkv = qkv_w @ xn + qkv_b                     # [3C, T]
    per head h (NH=8, ch=64): q,k,v; w = softmax((q*s)^T (k*s)); a = v @ w^T
    h = proj_w @ a + proj_b
    out = (xf + h) / sqrt(2)

Sharding: data-parallel over batch. 8 batch elements -> 8 NeuronCores, one
each. Weights replicated. No cross-core communication needed.

Device algorithm (v2, fp8 + pipelined):
  - QKV and attention-MM2 run in fp8e4 with DoubleRow perf mode (2 rows of
    the contraction per PE cell -> half the matmul instructions). q/k stay
    bf16 for MM1 (64-row contraction cannot pack, fp8 wouldn't be faster).
    proj stays bf16 for accuracy (it feeds the residual directly).
  - GroupNorm statistics stream behind the x DMA chunk-by-chunk; xn is
    written split across ScalarE (Identity w/ per-channel scale+bias) and
    VectorE so QKV can start ~9us in.
  - A burst of dummy matmuls at t=0 burns the HAM cold-start window so real
    matmuls run at 2.4 GHz from the first QKV tile.
  - Attention softmax: w^T[s,t] layout; exp tiles are split between ScalarE
    (table exp -> fp8 out) and VectorE (Schraudolph bit-trick exp: one
    tensor_scalar writing int8 bits of fp8e4). The softmax denominator
    comes for free from 64 ones-columns appended to v^T in MM2.
  - MM2 epilogue: denominators for both half-pairs are copied into one
    [128,512] tile, one reciprocal_approx_fast per t-chunk, then the
    normalizing multiplies write bf16 a (the fp8 v-scale lambda cancels in
    the num/den ratio).
  - Weight scale lambda=16 on wq/wk/wv keeps fp8 weights out of the
    subnormal range; the q*k logit scale (QK_SCALE^2 * lambda^-2 = 1/2048)
    is folded into the exp's affine pre-scale.
"""

import ml_dtypes
import numpy as np

import concourse.bass as bass
import concourse.mybir as mybir
import concourse.tile as tile
from concourse import bacc
from concourse.bass_utils import run_bass_kernel_spmd

B, C, T = 8, 512, 1024
NH, CH, G = 8, 64, 32
GS = C // G  # 16 channels per group
EPS = 1e-6
NCORES = 8
P = 128
KC = C // P   # 4 chunks of 128 channels
SCN = T // P  # 8 s-chunks
SCP = SCN // 2  # 4 s-chunk pairs (DoubleRow)
NT = T // 512   # 2 t-chunks of 512
ISQ2 = float(1.0 / np.sqrt(2.0))
QK_SCALE2 = float(1.0 / np.sqrt(CH))  # (1/sqrt(sqrt(ch)))^2
LAM = 16.0  # fp8 scale for wq/wk/wv (keeps weights out of subnormals)
EXP_SCALE = QK_SCALE2 / (LAM * LAM)   # = 1/2048
LOG2E = 1.4426950408889634
SCH_C = 0.05  # Schraudolph bias correction (device cast = round-to-nearest)

F32 = mybir.dt.float32
BF16 = mybir.dt.bfloat16
FP8 = mybir.dt.float8e4
I8 = mybir.dt.int8

N_DUMMY = 16  # HAM warm-up matmuls at t=0
# exp tiles handed to VectorE (Schraudolph) per pair; rest go to ScalarE
EXPS_DVE = {(1, 1), (3, 1), (4, 0), (5, 1), (7, 1)}

_GRAPH_CACHE = {}


def _build_graph(qkv_bias_nz: bool, proj_bias_nz: bool, debug_taps: bool = False):
    nc = bacc.Bacc("TRN2", target_bir_lowering=False, debug=False)
    AF = mybir.ActivationFunctionType

    # ---- DRAM I/O ------------------------------------------------------
    x_d = nc.dram_tensor("x", [C, T], F32, kind="ExternalInput").ap()
    wq_d = nc.dram_tensor("wqT", [C, C], FP8, kind="ExternalInput").ap()
    wk_d = nc.dram_tensor("wkT", [C, C], FP8, kind="ExternalInput").ap()
    wv_d = nc.dram_tensor("wvT", [C, C], FP8, kind="ExternalInput").ap()
    pw_d = nc.dram_tensor("pwT", [C, C], BF16, kind="ExternalInput").ap()
    gnw_d = nc.dram_tensor("gnw", [C], F32, kind="ExternalInput").ap()
    gnb_d = nc.dram_tensor("gnb", [C], F32, kind="ExternalInput").ap()
    ind16_d = nc.dram_tensor("ind16", [C, G], F32, kind="ExternalInput").ap()
    indT_d = nc.dram_tensor("indT", [G, C], F32, kind="ExternalInput").ap()
    qb_d = kb_d = vb_d = pb_d = None
    if qkv_bias_nz:
        qb_d = nc.dram_tensor("qb", [C], F32, kind="ExternalInput").ap()
        kb_d = nc.dram_tensor("kb", [C], F32, kind="ExternalInput").ap()
        vb_d = nc.dram_tensor("vb", [C], F32, kind="ExternalInput").ap()
    if proj_bias_nz:
        pb_d = nc.dram_tensor("pb", [C], F32, kind="ExternalInput").ap()
    out_d = nc.dram_tensor("out", [C, T], F32, kind="ExternalOutput").ap()
    dbg = {}
    if debug_taps:
        dbg["xn"] = nc.dram_tensor("dbg_xn", [P, KC, T], mybir.dt.uint8, kind="ExternalOutput").ap()
        dbg["q"] = nc.dram_tensor("dbg_q", [P, KC, T], BF16, kind="ExternalOutput").ap()
        dbg["k"] = nc.dram_tensor("dbg_k", [P, KC, T], BF16, kind="ExternalOutput").ap()
        dbg["vt"] = nc.dram_tensor("dbg_vt", [P, SCN, NH * P], mybir.dt.uint8, kind="ExternalOutput").ap()
        dbg["ew0"] = nc.dram_tensor("dbg_ew0", [SCN, P, T], mybir.dt.uint8, kind="ExternalOutput").ap()
        dbg["a"] = nc.dram_tensor("dbg_a", [P, KC, T], BF16, kind="ExternalOutput").ap()

    with tile.TileContext(nc) as tc:
        with (
            tc.tile_pool(name="big", bufs=1) as big,
            tc.tile_pool(name="wpool", bufs=1) as wpool,
            tc.tile_pool(name="small", bufs=1) as small,
            tc.tile_pool(name="ew", bufs=16) as ewpool,
            tc.tile_pool(name="dn", bufs=2) as dnpool,
            tc.tile_pool(name="ps1", bufs=2, space="PSUM") as ps1,
            tc.tile_pool(name="ps2", bufs=4, space="PSUM") as ps2,
        ):
            # ---- phase 0: warmups + loads -----------------------------
            # Dummy matmuls burn the HAM cold window while DMAs land.
            dmy_sb = small.tile([P, 640], BF16, tag="dmy")
            nc.vector.memset(dmy_sb, 0.0)
            pdmy = ps2.tile([P, 512], F32, tag="ps2")
            for _ in range(N_DUMMY):
                nc.tensor.matmul(
                    pdmy, lhsT=dmy_sb[:, 0:128], rhs=dmy_sb[:, 128:640],
                    start=True, stop=True,
                )

            # Warm the Ln/Exp table set (one ACT_TABLE_LOAD total).
            warm = small.tile([G, 1], F32, tag="warm")
            nc.vector.memset(warm, 1.0)
            nc.scalar.activation(out=warm, in_=warm, func=AF.Ln)

            # x first (GroupNorm is the critical path), then weights.
            x_sb = big.tile([P, KC, T], F32, tag="x")
            x_dr = x_d.rearrange("(o p) t -> p o t", p=P)
            for o in range(KC):
                for hlf in range(2):
                    nc.sync.dma_start(
                        out=x_sb[:, o, hlf * 512:(hlf + 1) * 512],
                        in_=x_dr[:, o, hlf * 512:(hlf + 1) * 512],
                    )

            wq_sb = wpool.tile([P, KC, C], FP8, tag="wq")
            nc.sync.dma_start(out=wq_sb, in_=wq_d.rearrange("(o p) n -> p o n", p=P))
            wk_sb = wpool.tile([P, KC, C], FP8, tag="wk")
            nc.sync.dma_start(out=wk_sb, in_=wk_d.rearrange("(o p) n -> p o n", p=P))
            wv_sb = wpool.tile([P, KC, C], FP8, tag="wv")
            nc.sync.dma_start(out=wv_sb, in_=wv_d.rearrange("(o p) n -> p o n", p=P))
            pw_sb = wpool.tile([P, KC, C], BF16, tag="pw")
            nc.sync.dma_start(out=pw_sb, in_=pw_d.rearrange("(o p) n -> p o n", p=P))

            gnw_sb = small.tile([P, KC], F32, tag="gnw")
            nc.sync.dma_start(out=gnw_sb, in_=gnw_d.rearrange("(o p) -> p o", p=P))
            gnb_sb = small.tile([P, KC], F32, tag="gnb")
            nc.sync.dma_start(out=gnb_sb, in_=gnb_d.rearrange("(o p) -> p o", p=P))
            ind16_sb = small.tile([P, KC, G], F32, tag="ind16")
            nc.sync.dma_start(
                out=ind16_sb, in_=ind16_d.rearrange("(o p) g -> p o g", p=P)
            )
            indT_sb = small.tile([G, KC, P], F32, tag="indT")
            nc.sync.dma_start(out=indT_sb, in_=indT_d.rearrange("g (o p) -> g o p", p=P))

            bias_aps = {}
            for nm, d in (("qb", qb_d), ("kb", kb_d), ("pb", pb_d)):
                if d is not None:
                    t_ = small.tile([P, KC], F32, tag=nm)
                    nc.sync.dma_start(out=t_, in_=d.rearrange("(o p) -> p o", p=P))
                    bias_aps[nm] = t_
            if vb_d is not None:
                vb_bc = small.tile([P, C], F32, tag="vb")
                nc.sync.dma_start(
                    out=vb_bc,
                    in_=bass.AP(tensor=vb_d.tensor, offset=vb_d.offset,
                                ap=[[0, P]] + vb_d.ap),
                )
                bias_aps["vb"] = vb_bc

            # v^T augmented: per head 128 cols = [64 v^T cols | 64 lambda].
            # MM2 rows 64:128 then hold lambda*sum(ew) -> the lambda on the
            # v columns cancels in the num/den ratio.
            vT_sb = big.tile([P, SCN, NH * P], FP8, tag="vT")
            vT4 = vT_sb.rearrange("p s (h z) -> p s h z", z=P)
            nc.gpsimd.memset(vT4[:, :, :, CH:P], LAM)

            # ---- phase 1: GroupNorm (streams behind the x DMA) --------
            stats6 = small.tile([P, KC, 2, 6], F32, tag="stats6")
            mv = small.tile([P, KC, 2], F32, tag="mv")
            stats2 = small.tile([P, KC, 2], F32, tag="stats2")
            psum_s = ps2.tile([G, 2], F32, tag="ps2")
            for o in range(KC):
                for hlf in range(2):
                    nc.vector.bn_stats(
                        out=stats6[:, o, hlf, :],
                        in_=x_sb[:, o, hlf * 512:(hlf + 1) * 512],
                    )
                nc.vector.bn_aggr(out=mv[:, o, :], in_=stats6[:, o, :, :])
                # stats2 = (mean, E[x^2]) per channel
                nc.vector.tensor_copy(out=stats2[:, o, 0:1], in_=mv[:, o, 0:1])
                nc.vector.tensor_mul(
                    out=stats2[:, o, 1:2], in0=mv[:, o, 0:1], in1=mv[:, o, 0:1]
                )
                nc.vector.tensor_add(
                    out=stats2[:, o, 1:2], in0=stats2[:, o, 1:2], in1=mv[:, o, 1:2]
                )
                # group reduce accumulates chunk-by-chunk (ind16 holds 1/16)
                nc.tensor.matmul(
                    psum_s,
                    lhsT=ind16_sb[:, o, :],
                    rhs=stats2[:, o, :],
                    start=(o == 0),
                    stop=(o == KC - 1),
                )

            musd = small.tile([G, 2], F32, tag="musd")
            nc.vector.tensor_copy(out=musd, in_=psum_s)  # (mu, E[x^2])
            varg = small.tile([G, 1], F32, tag="varg")
            nc.vector.tensor_mul(out=varg, in0=musd[:, 0:1], in1=musd[:, 0:1])
            nc.vector.tensor_sub(out=varg, in0=musd[:, 1:2], in1=varg)
            eps_sb = small.tile([G, 1], F32, tag="eps")
            nc.vector.memset(eps_sb, EPS)
            # rstd = exp(-0.5*ln(var+eps)); Ln/Exp share the warmed set
            nc.scalar.activation(out=varg, in_=varg, func=AF.Ln, bias=eps_sb)
            nc.scalar.activation(out=musd[:, 1:2], in_=varg, func=AF.Exp, scale=-0.5)

            # broadcast (mu, rstd) back to channels: one PSUM tile, 4 tiny MMs
            psum_b = ps2.tile([P, KC * 2], F32, tag="ps2")
            for o in range(KC):
                nc.tensor.matmul(
                    psum_b[:, o * 2:(o + 1) * 2], lhsT=indT_sb[:, o, :], rhs=musd,
                    start=True, stop=True,
                )
            musd_c = small.tile([P, KC, 2], F32, tag="musd_c")
            nc.vector.tensor_copy(out=musd_c, in_=psum_b.rearrange("p (o c) -> p o c", c=2))

            # A = rstd * gn_w ; Bq = gn_b - mu * A   (strided, one op each)
            A_sb = small.tile([P, KC], F32, tag="A")
            B_sb = small.tile([P, KC], F32, tag="B")
            nc.vector.tensor_mul(out=A_sb, in0=musd_c[:, :, 1], in1=gnw_sb)
            nc.vector.tensor_mul(out=B_sb, in0=musd_c[:, :, 0], in1=A_sb)
            nc.vector.tensor_sub(out=B_sb, in0=gnb_sb, in1=B_sb)

            # xn = x*A + B -> fp8; chunks 0/1 on ScalarE, 2/3 on VectorE
            xn_sb = big.tile([P, KC, T], FP8, tag="xn")
            for o in range(KC):
                if o < 2:
                    nc.scalar.activation(
                        out=xn_sb[:, o, :], in_=x_sb[:, o, :], func=AF.Identity,
                        bias=B_sb[:, o:o + 1], scale=A_sb[:, o:o + 1],
                    )
                else:
                    nc.vector.tensor_scalar(
                        out=xn_sb[:, o, :], in0=x_sb[:, o, :],
                        scalar1=A_sb[:, o:o + 1], scalar2=B_sb[:, o:o + 1],
                        op0=mybir.AluOpType.mult, op1=mybir.AluOpType.add,
                    )

            # ---- phase 2: QKV projections (fp8 DoubleRow) -------------
            q_sb = big.tile([P, KC, T], BF16, tag="q")
            k_sb = big.tile([P, KC, T], BF16, tag="k")

            def emit_qk(j):
                for dst, w_sb, bias in (
                    (q_sb, wq_sb, bias_aps.get("qb")),
                    (k_sb, wk_sb, bias_aps.get("kb")),
                ):
                    for t in range(NT):
                        pq = ps2.tile([P, 512], F32, tag="ps2")
                        for kk in range(KC // 2):
                            nc.tensor.matmul(
                                pq,
                                lhsT=w_sb[:, 2 * kk:2 * kk + 2, j * P:(j + 1) * P],
                                rhs=xn_sb[:, 2 * kk:2 * kk + 2, t * 512:(t + 1) * 512],
                                start=(kk == 0),
                                stop=(kk == KC // 2 - 1),
                                perf_mode=mybir.MatmulPerfMode.DoubleRow,
                            )
                        dslice = dst[:, j, t * 512:(t + 1) * 512]
                        if bias is not None:
                            nc.vector.tensor_scalar(
                                out=dslice, in0=pq, scalar1=bias[:, j:j + 1],
                                scalar2=None, op0=mybir.AluOpType.add,
                            )
                        else:
                            nc.vector.tensor_copy(out=dslice, in_=pq)

            def emit_v():
                for sc in range(SCN):
                    pv = ps2.tile([P, 512], F32, tag="ps2")
                    for kk in range(KC // 2):
                        nc.tensor.matmul(
                            pv,
                            lhsT=xn_sb[:, 2 * kk:2 * kk + 2, sc * P:(sc + 1) * P],
                            rhs=wv_sb[:, 2 * kk:2 * kk + 2, :],
                            start=(kk == 0),
                            stop=(kk == KC // 2 - 1),
                            perf_mode=mybir.MatmulPerfMode.DoubleRow,
                        )
                    vdst = vT4[:, sc, :, 0:CH]  # [P, NH, CH] strided dst
                    if "vb" in bias_aps:
                        nc.vector.scalar_tensor_tensor(
                            out=vdst, in0=pv.rearrange("p (h z) -> p h z", z=CH),
                            scalar=0.0,
                            in1=bias_aps["vb"].rearrange("p (h z) -> p h z", z=CH),
                            op0=mybir.AluOpType.add, op1=mybir.AluOpType.add,
                        )
                    else:
                        # ScalarE copy: VectorE is loaded with GN + casts here
                        nc.scalar.copy(
                            out=vdst, in_=pv.rearrange("p (h z) -> p h z", z=CH)
                        )

            # ---- attention helpers ------------------------------------
            a_sb = big.tile([P, KC, T], BF16, tag="a")
            SCH_A = float(8.0 * LOG2E * EXP_SCALE)
            SCH_B = float(8.0 * (7.0 - SCH_C))

            def emit_mm1_exps(j, ew, mm2_iter):
                """MM1 + exp for pair j; MM2 groups of pair j-1 interleaved
                after every other sc so the PE has fill work while exps of
                this pair drain the ps1 ring."""
                for sc in range(SCN):
                    ptiles = {}
                    for hb in range(2):
                        h0 = hb * CH
                        pw1 = ps1.tile([P, T], F32, tag="mm1")
                        for t in range(NT):
                            nc.tensor.matmul(
                                pw1[:, t * 512:(t + 1) * 512],
                                lhsT=k_sb[h0:h0 + CH, j, sc * P:(sc + 1) * P],
                                rhs=q_sb[h0:h0 + CH, j, t * 512:(t + 1) * 512],
                                start=True,
                                stop=True,
                            )
                        ptiles[hb] = pw1
                    scp, half = sc // 2, sc % 2
                    for hb in range(2):
                        if (scp, hb) not in ew:
                            et = ewpool.tile([P, 2, T], FP8, tag="ew")
                            ew[(scp, hb)] = et
                        et = ew[(scp, hb)]
                        if (sc, hb) in EXPS_DVE:
                            nc.vector.tensor_scalar(
                                out=et[:, half, :].bitcast(I8), in0=ptiles[hb],
                                scalar1=SCH_A, scalar2=SCH_B,
                                op0=mybir.AluOpType.mult, op1=mybir.AluOpType.add,
                            )
                        else:
                            nc.scalar.activation(
                                out=et[:, half, :], in_=ptiles[hb],
                                func=AF.Exp, scale=float(EXP_SCALE),
                            )
                        if debug_taps and j == 0 and hb == 0:
                            nc.sync.dma_start(
                                out=dbg["ew0"][sc],
                                in_=et[:, half, :].bitcast(mybir.dt.uint8),
                            )
                    if mm2_iter is not None and sc % 2 == 1:
                        next(mm2_iter, None)

            def mm2_groups(j, ew):
                """Generator: one MM2 group (+t-epilogue when ready) per next()."""
                pa = {}
                for t in range(NT):
                    for hb in range(2):
                        h = 2 * j + hb
                        pt = ps2.tile([P, 512], F32, tag="ps2")
                        for scp in range(SCP):
                            nc.tensor.matmul(
                                pt,
                                lhsT=vT_sb[:, 2 * scp:2 * scp + 2, h * P:(h + 1) * P],
                                rhs=ew[(scp, hb)][:, :, t * 512:(t + 1) * 512],
                                start=(scp == 0),
                                stop=(scp == SCP - 1),
                                perf_mode=mybir.MatmulPerfMode.DoubleRow,
                            )
                        pa[hb] = pt
                        if hb == 1:
                            # epilogue for this t: stack both denominators,
                            # one reciprocal, two normalizing muls -> a(bf16)
                            dd = dnpool.tile([P, 512], F32, tag="dd")
                            nc.vector.tensor_copy(out=dd[0:CH, :], in_=pa[0][CH:2 * CH, :])
                            nc.vector.tensor_copy(out=dd[CH:P, :], in_=pa[1][CH:2 * CH, :])
                            rr = dnpool.tile([P, 512], F32, tag="rr")
                            nc.vector.reciprocal_approx_fast(out=rr, in_=dd)
                            for hb2 in range(2):
                                nc.vector.tensor_mul(
                                    out=a_sb[hb2 * CH:(hb2 + 1) * CH, j,
                                             t * 512:(t + 1) * 512],
                                    in0=pa[hb2][0:CH, :],
                                    in1=rr[hb2 * CH:(hb2 + 1) * CH, :],
                                )
                        yield

            def drain(it):
                if it is not None:
                    for _ in it:
                        pass

            # ---- pipeline: QKV ahead, MM1/exp paced, MM2 trailing -----
            emit_qk(0)
            emit_v()
            emit_qk(1)
            mm2_prev = None
            for j in range(KC):
                ew_cur = {}
                emit_mm1_exps(j, ew_cur, mm2_prev)
                if j + 2 < KC:
                    emit_qk(j + 2)
                drain(mm2_prev)
                mm2_prev = mm2_groups(j, ew_cur)
            drain(mm2_prev)

            if debug_taps:
                nc.sync.dma_start(out=dbg["xn"], in_=xn_sb.bitcast(mybir.dt.uint8))
                nc.sync.dma_start(out=dbg["q"], in_=q_sb)
                nc.sync.dma_start(out=dbg["k"], in_=k_sb)
                nc.sync.dma_start(out=dbg["vt"], in_=vT_sb.bitcast(mybir.dt.uint8))
                nc.sync.dma_start(out=dbg["a"], in_=a_sb)

            # ---- output projection + residual (bf16 matmul) -----------
            out_sb = big.tile([P, KC, T], F32, tag="osb")
            for o in range(KC):
                for t in range(NT):
                    ph = ps2.tile([P, 512], F32, tag="ps2")
                    for k in range(KC):
                        nc.tensor.matmul(
                            ph,
                            lhsT=pw_sb[:, k, o * P:(o + 1) * P],
                            rhs=a_sb[:, k, t * 512:(t + 1) * 512],
                            start=(k == 0),
                            stop=(k == KC - 1),
                        )
                    if "pb" in bias_aps:
                        nc.vector.tensor_scalar(
                            out=ph, in0=ph, scalar1=bias_aps["pb"][:, o:o + 1],
                            scalar2=None, op0=mybir.AluOpType.add,
                        )
                    # out = x * (1/sqrt2) + h'   (1/sqrt2 folded into pwT/pb)
                    nc.vector.scalar_tensor_tensor(
                        out=out_sb[:, o, t * 512:(t + 1) * 512],
                        in0=x_sb[:, o, t * 512:(t + 1) * 512],
                        scalar=ISQ2,
                        in1=ph,
                        op0=mybir.AluOpType.mult,
                        op1=mybir.AluOpType.add,
                    )
                nc.sync.dma_start(
                    out=out_d.rearrange("(o p) t -> p o t", p=P)[:, o, :],
                    in_=out_sb[:, o, :],
                )

    nc.compile()
    return nc


def _host_prep(qkv_w, qkv_b, proj_w, proj_b):
    """Build the replicated (per-core-identical) weight/const arrays."""
    qkv_w = np.asarray(qkv_w, np.float32)
    qkv_b = np.asarray(qkv_b, np.float32)
    proj_w = np.asarray(proj_w, np.float32)
    proj_b = np.asarray(proj_b, np.float32)

    w3 = qkv_w.reshape(NH, 3 * CH, C)  # per head: rows 0:64 q, 64:128 k, 128:192 v
    b3 = qkv_b.reshape(NH, 3 * CH)
    wq = w3[:, 0:CH, :] * LAM               # [NH, CH, C]
    wk = w3[:, CH:2 * CH, :] * LAM
    wv = w3[:, 2 * CH:3 * CH, :] * LAM
    qb = (b3[:, 0:CH] * LAM).reshape(C)
    kb = (b3[:, CH:2 * CH] * LAM).reshape(C)
    vb = (b3[:, 2 * CH:3 * CH] * LAM).reshape(C)

    FP8NP = ml_dtypes.float8_e4m3
    wqT = np.ascontiguousarray(wq.reshape(C, C).T.astype(FP8NP))  # [C_in, NH*CH]
    wkT = np.ascontiguousarray(wk.reshape(C, C).T.astype(FP8NP))
    wvT = np.ascontiguousarray(wv.reshape(C, C).T.astype(FP8NP))
    pwT = np.ascontiguousarray((proj_w * ISQ2).T.astype(ml_dtypes.bfloat16))
    pb = proj_b * ISQ2

    ind16 = np.zeros((C, G), np.float32)
    ind16[np.arange(C), np.arange(C) // GS] = 1.0 / GS
    indT = np.zeros((G, C), np.float32)
    indT[np.arange(C) // GS, np.arange(C)] = 1.0

    return dict(
        wqT=wqT, wkT=wkT, wvT=wvT, pwT=pwT,
        qb=qb, kb=kb, vb=vb, pb=pb,
        ind16=ind16, indT=indT,
    )


def kernel(**inputs):
    x = np.asarray(inputs["x"], np.float32)
    gn_w = np.asarray(inputs["gn_w"], np.float32)
    gn_b = np.asarray(inputs["gn_b"], np.float32)
    qkv_b = np.asarray(inputs["qkv_b"], np.float32)
    proj_b = np.asarray(inputs["proj_b"], np.float32)

    prep = _host_prep(inputs["qkv_w"], qkv_b, inputs["proj_w"], proj_b)
    qkv_bias_nz = bool(np.any(qkv_b != 0))
    proj_bias_nz = bool(np.any(proj_b != 0))

    key = (qkv_bias_nz, proj_bias_nz)
    if key not in _GRAPH_CACHE:
        _GRAPH_CACHE[key] = _build_graph(qkv_bias_nz, proj_bias_nz)
    nc = _GRAPH_CACHE[key]

    shared = dict(
        wqT=prep["wqT"], wkT=prep["wkT"], wvT=prep["wvT"], pwT=prep["pwT"],
        gnw=gn_w, gnb=gn_b, ind16=prep["ind16"], indT=prep["indT"],
    )
    if qkv_bias_nz:
        shared.update(qb=prep["qb"], kb=prep["kb"], vb=prep["vb"])
    if proj_bias_nz:
        shared.update(pb=prep["pb"])

    in_maps = [
        {**shared, "x": np.ascontiguousarray(x[i].reshape(C, T))}
        for i in range(NCORES)
    ]
    res = run_bass_kernel_spmd(nc, in_maps, core_ids=list(range(NCORES)))
    out = np.stack(
        [res.results[i]["out"].reshape(C, 32, 32) for i in range(NCORES)]
    )
    kernel._last_results = res
    return out
